# revision 16
# baseline (speedup 1.0000x reference)
"""BDH model (embed -> proj -> Hebbian memory -> k-WTA -> logits) on 8 TRN2 cores.

Sharding: data-parallel over N=4096 tokens (512/core) for embed/proj/topk;
AllGather of (transposed, bf16) h_sparse; ReduceScatter of the Hebbian
delta; vocab-sharded (4000 cols/core) logits GEMM in bf16.
"""

import sys

sys.path.insert(0, "/opt/trn_rl_repo")

import numpy as np
import ml_dtypes

VOCAB, EMB, HID = 32000, 512, 1024
B, S = 2, 2048
N = B * S  # 4096 tokens
NCORES = 8
NSH = N // NCORES  # 512 rows per core
VSH = VOCAB // NCORES  # 4000 vocab cols per core
HSH = HID // NCORES  # 128 hid rows per core (W_new shard)
K = 32
LR = 0.01
DECAY = 0.001
NEG = -1e30

_cache = {}


def build(has_bout: bool):
    import concourse.bass as bass
    import concourse.bacc as bacc
    import concourse.mybir as mybir
    import concourse.tile as tile

    f32 = mybir.dt.float32
    bf16 = mybir.dt.bfloat16
    Alu = mybir.AluOpType
    Act = mybir.ActivationFunctionType

    nc = bacc.Bacc(None, num_devices=NCORES)

    # ---- I/O ----
    x_embT = nc.declare_dram_parameter("x_embT", [EMB, NSH], f32, isOutput=False)
    w_inT = nc.declare_dram_parameter("w_inT", [EMB, HID], f32, isOutput=False)
    w_memT = nc.declare_dram_parameter("w_memT", [HID, HID], f32, isOutput=False)
    wmem_rows = nc.declare_dram_parameter("wmem_rows", [HSH, HID], f32, isOutput=False)
    woutT = nc.declare_dram_parameter("woutT", [HID, VSH], bf16, isOutput=False)
    b_in2d = nc.declare_dram_parameter("b_in2d", [HID, 1], f32, isOutput=False)
    identb = nc.declare_dram_parameter("identb", [128, 128], bf16, isOutput=False)
    identf = nc.declare_dram_parameter("identf", [128, 128], f32, isOutput=False)
    if has_bout:
        bout_bc = nc.declare_dram_parameter("bout_bc", [128, VSH], f32, isOutput=False)
    logits_out = nc.declare_dram_parameter("logits_out", [N, VSH], f32, isOutput=True)
    hsp_out = nc.declare_dram_parameter("hsp_out", [NSH, HID], f32, isOutput=True)
    wnew_out = nc.declare_dram_parameter("wnew_out", [HSH, HID], f32, isOutput=True)

    rg = [list(range(NCORES))]

    with tile.TileContext(nc) as tc:
        # long-lived pools
        with (
            tc.tile_pool(name="consts", bufs=1) as consts,
            tc.tile_pool(name="carry", bufs=1) as carry,
            tc.tile_pool(name="dram", bufs=1, space="DRAM") as dram,
        ):
            identb_sb = consts.tile([128, 128], bf16)
            identf_sb = consts.tile([128, 128], f32)
            bin_sb = consts.tile([128, 8], f32)
            nc.sync.dma_start(identb_sb[:, :], identb[:, :])
            nc.sync.dma_start(identf_sb[:, :], identf[:, :])
            nc.sync.dma_start(
                bin_sb[:, :], b_in2d[:, :].rearrange("(a p) o -> p (a o)", p=128)
            )

            h_inT_sb = carry.tile([128, 8, NSH], f32)  # h-major, 16K/part
            hsp_bf = carry.tile([128, 4, HID], bf16)  # n-major, 8K/part
            h_in_n = carry.tile([128, 4, HID], bf16)  # n-major, 8K/part
            woutT_sb = carry.tile([128, 8, VSH], bf16)  # 64K/part
            nc.sync.dma_start(
                woutT_sb[:, :, :], woutT[:, :].rearrange("(a p) v -> p a v", p=128)
            )

            # one allgather per 128-token chunk so logits can start early
            hsT_dram = [dram.tile([HID, 128], bf16, name=f"hsTd{i}") for i in range(4)]
            ag_dram = [
                dram.tile([NCORES, HID, 128], bf16, addr_space="Shared", name=f"ag{i}")
                for i in range(4)
            ]
            delta_dram = dram.tile([HID, HID], bf16)
            rs_dram = dram.tile([HSH, HID], bf16)

            # ---------------- phases A-C ----------------
            with (
                tc.tile_pool(name="w1", bufs=1) as w1,
                tc.tile_pool(name="wmem", bufs=1) as wmemp,
                tc.tile_pool(name="topk", bufs=1) as topk,
                tc.tile_pool(name="ps1", bufs=1, space="PSUM") as ps1,
            ):
                xemb_sb = w1.tile([128, 4, NSH], f32)
                winT_sb = w1.tile([128, 4, HID], f32)
                wmemT_sb = wmemp.tile([128, 8, HID], f32)
                hsT_own = w1.tile([128, 8, NSH], bf16)  # h-major, 8K/part
                nc.sync.dma_start(
                    xemb_sb[:, :, :], x_embT[:, :].rearrange("(a p) n -> p a n", p=128)
                )
                nc.sync.dma_start(
                    winT_sb[:, :, :], w_inT[:, :].rearrange("(a p) h -> p a h", p=128)
                )
                nc.sync.dma_start(
                    wmemT_sb[:, :, :], w_memT[:, :].rearrange("(a p) j -> p a j", p=128)
                )

                # B: h_inT[h, n] = relu(W_in @ x + b), h-major
                for hm in range(8):
                    psA = ps1.tile([128, 512], f32, tag="psA", bufs=2)
                    for ec in range(4):
                        nc.tensor.matmul(
                            psA[:, :],
                            lhsT=winT_sb[:, ec, hm * 128 : (hm + 1) * 128],
                            rhs=xemb_sb[:, ec, :],
                            start=(ec == 0),
                            stop=(ec == 3),
                        )
                    nc.scalar.activation(
                        h_inT_sb[:, hm, :],
                        psA[:, :],
                        Act.Relu,
                        bias=bin_sb[:, hm : hm + 1],
                    )

                # C: per 128-token tile: h_mem, top-32 threshold, h_sparse
                for m in range(4):
                    hmem = topk.tile([128, HID], f32, tag="hmem", bufs=2)
                    for jn in range(2):
                        psB = ps1.tile([128, 512], f32, tag="psB", bufs=4)
                        for hc in range(8):
                            nc.tensor.matmul(
                                psB[:, :],
                                lhsT=h_inT_sb[:, hc, m * 128 : (m + 1) * 128],
                                rhs=wmemT_sb[:, hc, jn * 512 : (jn + 1) * 512],
                                start=(hc == 0),
                                stop=(hc == 7),
                            )
                        nc.any.tensor_copy(hmem[:, jn * 512 : (jn + 1) * 512], psB[:, :])

                    m8a = topk.tile([128, 8], f32, tag="m8a", bufs=2)
                    m8b = topk.tile([128, 8], f32, tag="m8b", bufs=2)
                    m8c = topk.tile([128, 8], f32, tag="m8c", bufs=2)
                    m8d = topk.tile([128, 8], f32, tag="m8d", bufs=2)
                    t1 = topk.tile([128, HID], f32, tag="t1", bufs=1)
                    t2 = topk.tile([128, HID], f32, tag="t2", bufs=1)
                    t3 = topk.tile([128, HID], f32, tag="t3", bufs=1)
                    nc.vector.max(m8a[:, :], hmem[:, :])
                    nc.vector.match_replace(t1[:, :], m8a[:, :], hmem[:, :], NEG)
                    nc.vector.max(m8b[:, :], t1[:, :])
                    nc.vector.match_replace(t2[:, :], m8b[:, :], t1[:, :], NEG)
                    nc.vector.max(m8c[:, :], t2[:, :])
                    nc.vector.match_replace(t3[:, :], m8c[:, :], t2[:, :], NEG)
                    nc.vector.max(m8d[:, :], t3[:, :])

                    hsp = topk.tile([128, HID], f32, tag="hsp", bufs=2)
                    # h_sparse = (h_mem >= kth) * h_mem
                    nc.vector.scalar_tensor_tensor(
                        hsp[:, :],
                        hmem[:, :],
                        m8d[:, 7:8],
                        hmem[:, :],
                        op0=Alu.is_ge,
                        op1=Alu.mult,
                    )
                    nc.sync.dma_start(hsp_out[m * 128 : (m + 1) * 128, :], hsp[:, :])
                    nc.any.tensor_copy(hsp_bf[:, m, :], hsp[:, :])
                    # transpose own h_sparse (bf16) for the allgather
                    for hc in range(8):
                        psT = ps1.tile([128, 128], bf16, tag="psT", bufs=2)
                        nc.tensor.transpose(
                            psT[:, :],
                            hsp_bf[:, m, hc * 128 : (hc + 1) * 128],
                            identb_sb[:, :],
                        )
                        nc.any.tensor_copy(
                            hsT_own[:, hc, m * 128 : (m + 1) * 128], psT[:, :]
                        )
                    # D: ship this 128-token chunk out immediately
                    nc.sync.dma_start(
                        hsT_dram[m][:, :].rearrange("(a p) n -> p a n", p=128),
                        hsT_own[:, :, m * 128 : (m + 1) * 128],
                    )
                    nc.gpsimd.collective_compute(
                        "AllGather",
                        Alu.bypass,
                        replica_groups=rg,
                        ins=[hsT_dram[m].opt()],
                        outs=[ag_dram[m].opt()],
                    )

            # ---------------- phases E-F ----------------
            with (
                tc.tile_pool(name="big", bufs=1) as big,
                tc.tile_pool(name="late", bufs=1) as late,
                tc.tile_pool(name="ps2", bufs=1, space="PSUM") as ps2,
            ):
                # layout: [p, nb 4, hc 8, c 8, n 128]
                hsT_full = big.tile([128, 4, 8, NCORES, 128], bf16)  # 64K/part
                if has_bout:
                    bout_sb = late.tile([128, VSH], f32)
                    nc.sync.dma_start(bout_sb[:, :], bout_bc[:, :])
                for nb in range(4):
                    for c in range(NCORES):
                        nc.sync.dma_start(
                            hsT_full[:, nb, :, c, :],
                            ag_dram[nb][c, :, :].rearrange("(a p) n -> p a n", p=128),
                        )

                # E: transpose h_inT -> n-major bf16; partial delta; W_new
                for hc in range(8):
                    for m in range(4):
                        psT2 = ps2.tile([128, 128], f32, tag="psT2", bufs=2)
                        nc.tensor.transpose(
                            psT2[:, :],
                            h_inT_sb[:, hc, m * 128 : (m + 1) * 128],
                            identf_sb[:, :],
                        )
                        nc.any.tensor_copy(
                            h_in_n[:, m, hc * 128 : (hc + 1) * 128], psT2[:, :]
                        )
                with tc.tile_pool(name="deltap", bufs=1) as dp:
                    delta_sb = dp.tile([128, 8, HID], bf16)  # 16K/part
                    for ic in range(8):
                        for jn in range(2):
                            psD = ps2.tile([128, 512], f32, tag="psD", bufs=2)
                            for m in range(4):
                                nc.tensor.matmul(
                                    psD[:, :],
                                    lhsT=hsp_bf[:, m, ic * 128 : (ic + 1) * 128],
                                    rhs=h_in_n[:, m, jn * 512 : (jn + 1) * 512],
                                    start=(m == 0),
                                    stop=(m == 3),
                                )
                            nc.any.tensor_copy(
                                delta_sb[:, ic, jn * 512 : (jn + 1) * 512], psD[:, :]
                            )
                    nc.sync.dma_start(
                        delta_dram[:, :].rearrange("(a p) j -> p a j", p=128),
                        delta_sb[:, :, :],
                    )
                nc.gpsimd.collective_compute(
                    "ReduceScatter",
                    Alu.add,
                    replica_groups=rg,
                    ins=[delta_dram.opt()],
                    outs=[rs_dram.opt()],
                )
                rs_sb = late.tile([128, HID], bf16)
                wmr_sb = late.tile([128, HID], f32)
                rs_f = late.tile([128, HID], f32)
                wnew_sb = late.tile([128, HID], f32)
                nc.sync.dma_start(rs_sb[:, :], rs_dram[:, :])
                nc.sync.dma_start(wmr_sb[:, :], wmem_rows[:, :])
                nc.vector.tensor_copy(rs_f[:, :], rs_sb[:, :])
                nc.vector.tensor_scalar_mul(wnew_sb[:, :], wmr_sb[:, :], 1.0 - DECAY)
                # W_new = W_mem*(1-d) + (LR*(1-d)/N) * delta_sum
                nc.vector.scalar_tensor_tensor(
                    wnew_sb[:, :],
                    rs_f[:, :],
                    float(LR * (1.0 - DECAY) / N),
                    wnew_sb[:, :],
                    op0=Alu.mult,
                    op1=Alu.add,
                )
                nc.sync.dma_start(wnew_out[:, :], wnew_sb[:, :])

                # F: logits[n, v_shard] in bf16; chunk nb outer so early
                # allgather chunks unblock their 8 row-tiles first
                with tc.tile_pool(name="stage", bufs=1) as stagep:
                    for nb in range(4):
                        for c in range(NCORES):
                            mt = c * 4 + nb
                            st = stagep.tile([128, VSH], f32, tag="st", bufs=2)
                            for vc in range(8):
                                psL = ps2.tile([128, 512], f32, tag="psL", bufs=4)
                                for hc in range(8):
                                    nc.tensor.matmul(
                                        psL[:, :500],
                                        lhsT=hsT_full[:, nb, hc, c, :],
                                        rhs=woutT_sb[:, hc, vc * 500 : (vc + 1) * 500],
                                        start=(hc == 0),
                                        stop=(hc == 7),
                                    )
                                dst = st[:, vc * 500 : (vc + 1) * 500]
                                if has_bout:
                                    nc.vector.tensor_add(
                                        dst,
                                        psL[:, :500],
                                        bout_sb[:, vc * 500 : (vc + 1) * 500],
                                    )
                                elif vc % 2 == 0:
                                    nc.vector.tensor_copy(dst, psL[:, :500])
                                else:
                                    nc.scalar.copy(dst, psL[:, :500])
                            nc.sync.dma_start(
                                logits_out[mt * 128 : (mt + 1) * 128, :], st[:, :]
                            )
    nc.finalize()
    return nc


def _prep(tokens, emb_table, W_in, b_in, W_mem, W_out, b_out):
    f32 = np.float32
    bf16 = ml_dtypes.bfloat16
    tokens_flat = np.asarray(tokens).reshape(-1)
    x_emb = np.asarray(emb_table, f32)[tokens_flat]  # [N, EMB]
    w_inT = np.ascontiguousarray(np.asarray(W_in, f32).T)  # [EMB, HID]
    w_memT = np.ascontiguousarray(np.asarray(W_mem, f32).T)  # [HID, HID]
    woutT_full = np.asarray(W_out, f32).T.astype(bf16)  # [HID, VOCAB]
    b_in2d = np.asarray(b_in, f32).reshape(HID, 1).copy()
    identb = np.eye(128, dtype=bf16)
    identf = np.eye(128, dtype=f32)
    has_bout = bool(np.any(np.asarray(b_out) != 0))

    in_maps = []
    for c in range(NCORES):
        m = {
            "x_embT": np.ascontiguousarray(x_emb[c * NSH : (c + 1) * NSH].T),
            "w_inT": w_inT,
            "w_memT": w_memT,
            "wmem_rows": np.ascontiguousarray(
                np.asarray(W_mem, f32)[c * HSH : (c + 1) * HSH]
            ),
            "woutT": np.ascontiguousarray(woutT_full[:, c * VSH : (c + 1) * VSH]),
            "b_in2d": b_in2d,
            "identb": identb,
            "identf": identf,
        }
        if has_bout:
            m["bout_bc"] = np.ascontiguousarray(
                np.broadcast_to(
                    np.asarray(b_out, f32)[c * VSH : (c + 1) * VSH], (128, VSH)
                )
            )
        in_maps.append(m)
    return in_maps, has_bout


def run(inputs: dict, trace: bool = False):
    from concourse.bass_utils import run_bass_kernel_spmd

    in_maps, has_bout = _prep(**inputs)
    key = ("nc", has_bout)
    if key not in _cache:
        _cache[key] = build(has_bout)
    nc = _cache[key]
    res = run_bass_kernel_spmd(nc, in_maps, list(range(NCORES)), trace=trace)

    logits = np.concatenate([res.results[c]["logits_out"] for c in range(NCORES)], axis=1)
    h_sparse = np.concatenate(
        [res.results[c]["hsp_out"] for c in range(NCORES)], axis=0
    ).reshape(B, S, HID)
    w_new = np.concatenate([res.results[c]["wnew_out"] for c in range(NCORES)], axis=0)
    return (logits, h_sparse, w_new), res


def kernel(**inputs):
    outs, _ = run(inputs, trace=False)
    return outs


# revision 20
# speedup vs baseline: 1.1184x; 1.1184x over previous
"""BDH model (embed -> proj -> Hebbian memory -> k-WTA -> logits) on 8 TRN2 cores.

Sharding: data-parallel over N=4096 tokens (512/core) for embed/proj/topk;
AllGather of (transposed, bf16) h_sparse; ReduceScatter of the Hebbian
delta; vocab-sharded (4000 cols/core) logits GEMM in bf16.
"""

import sys

sys.path.insert(0, "/opt/trn_rl_repo")

import numpy as np
import ml_dtypes

VOCAB, EMB, HID = 32000, 512, 1024
B, S = 2, 2048
N = B * S  # 4096 tokens
NCORES = 8
NSH = N // NCORES  # 512 rows per core
VSH = VOCAB // NCORES  # 4000 vocab cols per core
HSH = HID // NCORES  # 128 hid rows per core (W_new shard)
K = 32
LR = 0.01
DECAY = 0.001
NEG = -1e30

_cache = {}


def build(has_bout: bool):
    import concourse.bass as bass
    import concourse.bacc as bacc
    import concourse.mybir as mybir
    import concourse.tile as tile

    f32 = mybir.dt.float32
    bf16 = mybir.dt.bfloat16
    Alu = mybir.AluOpType
    Act = mybir.ActivationFunctionType

    nc = bacc.Bacc(None, num_devices=NCORES)

    # ---- I/O ----
    x_embT = nc.declare_dram_parameter("x_embT", [EMB, NSH], f32, isOutput=False)
    w_inT = nc.declare_dram_parameter("w_inT", [EMB, HID], f32, isOutput=False)
    w_memT = nc.declare_dram_parameter("w_memT", [HID, HID], f32, isOutput=False)
    wmem_rows = nc.declare_dram_parameter("wmem_rows", [HSH, HID], f32, isOutput=False)
    woutT = nc.declare_dram_parameter("woutT", [HID, VSH], bf16, isOutput=False)
    b_in2d = nc.declare_dram_parameter("b_in2d", [HID, 1], f32, isOutput=False)
    identb = nc.declare_dram_parameter("identb", [128, 128], bf16, isOutput=False)
    identf = nc.declare_dram_parameter("identf", [128, 128], f32, isOutput=False)
    if has_bout:
        bout_bc = nc.declare_dram_parameter("bout_bc", [128, VSH], f32, isOutput=False)
    logits_out = nc.declare_dram_parameter("logits_out", [N, VSH], f32, isOutput=True)
    hsp_out = nc.declare_dram_parameter("hsp_out", [NSH, HID], f32, isOutput=True)
    wnew_out = nc.declare_dram_parameter("wnew_out", [HSH, HID], f32, isOutput=True)

    rg = [list(range(NCORES))]

    with tile.TileContext(nc) as tc:
        # long-lived pools
        with (
            tc.tile_pool(name="consts", bufs=1) as consts,
            tc.tile_pool(name="carry", bufs=1) as carry,
            tc.tile_pool(name="dram", bufs=1, space="DRAM") as dram,
        ):
            identb_sb = consts.tile([128, 128], bf16)
            identf_sb = consts.tile([128, 128], f32)
            bin_sb = consts.tile([128, 8], f32)
            nc.sync.dma_start(identb_sb[:, :], identb[:, :])
            nc.sync.dma_start(identf_sb[:, :], identf[:, :])
            nc.sync.dma_start(
                bin_sb[:, :], b_in2d[:, :].rearrange("(a p) o -> p (a o)", p=128)
            )

            h_inT_sb = carry.tile([128, 8, NSH], f32)  # h-major, 16K/part
            hsp_bf = carry.tile([128, 4, HID], bf16)  # n-major, 8K/part
            h_in_n = carry.tile([128, 4, HID], bf16)  # n-major, 8K/part
            woutT_sb = carry.tile([128, 8, VSH], bf16)  # 64K/part
            nc.sync.dma_start(
                woutT_sb[:, :, :], woutT[:, :].rearrange("(a p) v -> p a v", p=128)
            )

            # one allgather per 128-token chunk so logits can start early
            hsT_dram = [dram.tile([HID, 128], bf16, name=f"hsTd{i}") for i in range(4)]
            ag_dram = [
                dram.tile([NCORES, HID, 128], bf16, addr_space="Shared", name=f"ag{i}")
                for i in range(4)
            ]
            delta_dram = dram.tile([HID, HID], bf16)
            rs_dram = dram.tile([HSH, HID], bf16)

            # ---------------- phases A-C ----------------
            with (
                tc.tile_pool(name="w1", bufs=1) as w1,
                tc.tile_pool(name="wmem", bufs=1) as wmemp,
                tc.tile_pool(name="topk", bufs=1) as topk,
                tc.tile_pool(name="ps1", bufs=1, space="PSUM") as ps1,
            ):
                xemb_sb = w1.tile([128, 4, NSH], f32)
                winT_sb = w1.tile([128, 4, HID], f32)
                wmemT_sb = wmemp.tile([128, 8, HID], f32)
                hsT_own = w1.tile([128, 8, NSH], bf16)  # h-major, 8K/part
                nc.sync.dma_start(
                    xemb_sb[:, :, :], x_embT[:, :].rearrange("(a p) n -> p a n", p=128)
                )
                nc.sync.dma_start(
                    winT_sb[:, :, :], w_inT[:, :].rearrange("(a p) h -> p a h", p=128)
                )
                nc.sync.dma_start(
                    wmemT_sb[:, :, :], w_memT[:, :].rearrange("(a p) j -> p a j", p=128)
                )

                # B: h_inT[h, n] = relu(W_in @ x + b), h-major
                for hm in range(8):
                    psA = ps1.tile([128, 512], f32, tag="psA", bufs=2)
                    for ec in range(4):
                        nc.tensor.matmul(
                            psA[:, :],
                            lhsT=winT_sb[:, ec, hm * 128 : (hm + 1) * 128],
                            rhs=xemb_sb[:, ec, :],
                            start=(ec == 0),
                            stop=(ec == 3),
                        )
                    nc.scalar.activation(
                        h_inT_sb[:, hm, :],
                        psA[:, :],
                        Act.Relu,
                        bias=bin_sb[:, hm : hm + 1],
                    )

                # C: per 128-token tile: h_mem, top-32 threshold, h_sparse
                for m in range(4):
                    hmem = topk.tile([128, HID], f32, tag="hmem", bufs=2)
                    for jn in range(2):
                        psB = ps1.tile([128, 512], f32, tag="psB", bufs=4)
                        for hc in range(8):
                            nc.tensor.matmul(
                                psB[:, :],
                                lhsT=h_inT_sb[:, hc, m * 128 : (m + 1) * 128],
                                rhs=wmemT_sb[:, hc, jn * 512 : (jn + 1) * 512],
                                start=(hc == 0),
                                stop=(hc == 7),
                            )
                        nc.any.tensor_copy(hmem[:, jn * 512 : (jn + 1) * 512], psB[:, :])

                    m8a = topk.tile([128, 8], f32, tag="m8a", bufs=2)
                    m8b = topk.tile([128, 8], f32, tag="m8b", bufs=2)
                    m8c = topk.tile([128, 8], f32, tag="m8c", bufs=2)
                    m8d = topk.tile([128, 8], f32, tag="m8d", bufs=2)
                    t1 = topk.tile([128, HID], f32, tag="t1", bufs=1)
                    t2 = topk.tile([128, HID], f32, tag="t2", bufs=1)
                    t3 = topk.tile([128, HID], f32, tag="t3", bufs=1)
                    nc.vector.max(m8a[:, :], hmem[:, :])
                    nc.vector.match_replace(t1[:, :], m8a[:, :], hmem[:, :], NEG)
                    nc.vector.max(m8b[:, :], t1[:, :])
                    nc.vector.match_replace(t2[:, :], m8b[:, :], t1[:, :], NEG)
                    nc.vector.max(m8c[:, :], t2[:, :])
                    nc.vector.match_replace(t3[:, :], m8c[:, :], t2[:, :], NEG)
                    nc.vector.max(m8d[:, :], t3[:, :])

                    hsp = topk.tile([128, HID], f32, tag="hsp", bufs=2)
                    # h_sparse = (h_mem >= kth) * h_mem
                    nc.vector.scalar_tensor_tensor(
                        hsp[:, :],
                        hmem[:, :],
                        m8d[:, 7:8],
                        hmem[:, :],
                        op0=Alu.is_ge,
                        op1=Alu.mult,
                    )
                    nc.sync.dma_start(hsp_out[m * 128 : (m + 1) * 128, :], hsp[:, :])
                    nc.any.tensor_copy(hsp_bf[:, m, :], hsp[:, :])
                    # transpose own h_sparse (bf16) for the allgather
                    for hc in range(8):
                        psT = ps1.tile([128, 128], bf16, tag="psT", bufs=2)
                        nc.tensor.transpose(
                            psT[:, :],
                            hsp_bf[:, m, hc * 128 : (hc + 1) * 128],
                            identb_sb[:, :],
                        )
                        nc.any.tensor_copy(
                            hsT_own[:, hc, m * 128 : (m + 1) * 128], psT[:, :]
                        )
                    # D: ship this 128-token chunk out immediately
                    nc.sync.dma_start(
                        hsT_dram[m][:, :].rearrange("(a p) n -> p a n", p=128),
                        hsT_own[:, :, m * 128 : (m + 1) * 128],
                    )
                    nc.gpsimd.collective_compute(
                        "AllGather",
                        Alu.bypass,
                        replica_groups=rg,
                        ins=[hsT_dram[m].opt()],
                        outs=[ag_dram[m].opt()],
                    )

            # ---------------- phases E-F ----------------
            with (
                tc.tile_pool(name="big", bufs=1) as big,
                tc.tile_pool(name="late", bufs=1) as late,
                tc.tile_pool(name="ps2", bufs=1, space="PSUM") as ps2,
            ):
                # layout: [p, nb 4, hc 8, c 8, n 128]
                hsT_full = big.tile([128, 4, 8, NCORES, 128], bf16)  # 64K/part
                if has_bout:
                    bout_sb = late.tile([128, VSH], f32)
                    nc.sync.dma_start(bout_sb[:, :], bout_bc[:, :])
                for nb in range(4):
                    for c in range(NCORES):
                        nc.sync.dma_start(
                            hsT_full[:, nb, :, c, :],
                            ag_dram[nb][c, :, :].rearrange("(a p) n -> p a n", p=128),
                        )

                # E: transpose h_inT -> n-major bf16; partial delta; W_new
                psE_cm = tc.tile_pool(name="psE", bufs=1, space="PSUM")
                psE = psE_cm.__enter__()
                for hc in range(8):
                    for m in range(4):
                        psT2 = psE.tile([128, 128], f32, tag="psT2", bufs=2)
                        nc.tensor.transpose(
                            psT2[:, :],
                            h_inT_sb[:, hc, m * 128 : (m + 1) * 128],
                            identf_sb[:, :],
                        )
                        nc.any.tensor_copy(
                            h_in_n[:, m, hc * 128 : (hc + 1) * 128], psT2[:, :]
                        )
                with tc.tile_pool(name="deltap", bufs=1) as dp:
                    delta_sb = dp.tile([128, 8, HID], bf16)  # 16K/part
                    for ic in range(8):
                        for jn in range(2):
                            psD = psE.tile([128, 512], f32, tag="psD", bufs=2)
                            for m in range(4):
                                nc.tensor.matmul(
                                    psD[:, :],
                                    lhsT=hsp_bf[:, m, ic * 128 : (ic + 1) * 128],
                                    rhs=h_in_n[:, m, jn * 512 : (jn + 1) * 512],
                                    start=(m == 0),
                                    stop=(m == 3),
                                )
                            nc.any.tensor_copy(
                                delta_sb[:, ic, jn * 512 : (jn + 1) * 512], psD[:, :]
                            )
                    nc.sync.dma_start(
                        delta_dram[:, :].rearrange("(a p) j -> p a j", p=128),
                        delta_sb[:, :, :],
                    )
                psE_cm.__exit__(None, None, None)
                nc.gpsimd.collective_compute(
                    "ReduceScatter",
                    Alu.add,
                    replica_groups=rg,
                    ins=[delta_dram.opt()],
                    outs=[rs_dram.opt()],
                )
                rs_sb = late.tile([128, HID], bf16)
                wmr_sb = late.tile([128, HID], f32)
                rs_f = late.tile([128, HID], f32)
                wnew_sb = late.tile([128, HID], f32)
                nc.sync.dma_start(rs_sb[:, :], rs_dram[:, :])
                nc.sync.dma_start(wmr_sb[:, :], wmem_rows[:, :])
                nc.vector.tensor_copy(rs_f[:, :], rs_sb[:, :])
                nc.vector.tensor_scalar_mul(wnew_sb[:, :], wmr_sb[:, :], 1.0 - DECAY)
                # W_new = W_mem*(1-d) + (LR*(1-d)/N) * delta_sum
                nc.vector.scalar_tensor_tensor(
                    wnew_sb[:, :],
                    rs_f[:, :],
                    float(LR * (1.0 - DECAY) / N),
                    wnew_sb[:, :],
                    op0=Alu.mult,
                    op1=Alu.add,
                )
                nc.sync.dma_start(wnew_out[:, :], wnew_sb[:, :])

                # F: logits[n, v_shard] in bf16; chunk nb outer so early
                # allgather chunks unblock their 8 row-tiles first.
                # Results DMA straight from PSUM to DRAM (no staging copy).
                with tc.tile_pool(name="stage", bufs=1) as stagep:
                    for nb in range(4):
                        for c in range(NCORES):
                            mt = c * 4 + nb
                            for vc in range(8):
                                psL = ps2.tile([128, 512], f32, tag="psL", bufs=4)
                                for hc in range(8):
                                    nc.tensor.matmul(
                                        psL[:, :500],
                                        lhsT=hsT_full[:, nb, hc, c, :],
                                        rhs=woutT_sb[:, hc, vc * 500 : (vc + 1) * 500],
                                        start=(hc == 0),
                                        stop=(hc == 7),
                                    )
                                dst = logits_out[
                                    mt * 128 : (mt + 1) * 128, vc * 500 : (vc + 1) * 500
                                ]
                                st = stagep.tile([128, 500], f32, tag="st", bufs=8)
                                if has_bout:
                                    nc.vector.tensor_add(
                                        st[:, :],
                                        psL[:, :500],
                                        bout_sb[:, vc * 500 : (vc + 1) * 500],
                                    )
                                elif vc % 2 == 0:
                                    nc.vector.tensor_copy(st[:, :], psL[:, :500])
                                else:
                                    nc.scalar.copy(st[:, :], psL[:, :500])
                                nc.sync.dma_start(dst, st[:, :])
    nc.finalize()
    return nc


def _prep(tokens, emb_table, W_in, b_in, W_mem, W_out, b_out):
    f32 = np.float32
    bf16 = ml_dtypes.bfloat16
    tokens_flat = np.asarray(tokens).reshape(-1)
    x_emb = np.asarray(emb_table, f32)[tokens_flat]  # [N, EMB]
    w_inT = np.ascontiguousarray(np.asarray(W_in, f32).T)  # [EMB, HID]
    w_memT = np.ascontiguousarray(np.asarray(W_mem, f32).T)  # [HID, HID]
    woutT_full = np.asarray(W_out, f32).T.astype(bf16)  # [HID, VOCAB]
    b_in2d = np.asarray(b_in, f32).reshape(HID, 1).copy()
    identb = np.eye(128, dtype=bf16)
    identf = np.eye(128, dtype=f32)
    has_bout = bool(np.any(np.asarray(b_out) != 0))

    in_maps = []
    for c in range(NCORES):
        m = {
            "x_embT": np.ascontiguousarray(x_emb[c * NSH : (c + 1) * NSH].T),
            "w_inT": w_inT,
            "w_memT": w_memT,
            "wmem_rows": np.ascontiguousarray(
                np.asarray(W_mem, f32)[c * HSH : (c + 1) * HSH]
            ),
            "woutT": np.ascontiguousarray(woutT_full[:, c * VSH : (c + 1) * VSH]),
            "b_in2d": b_in2d,
            "identb": identb,
            "identf": identf,
        }
        if has_bout:
            m["bout_bc"] = np.ascontiguousarray(
                np.broadcast_to(
                    np.asarray(b_out, f32)[c * VSH : (c + 1) * VSH], (128, VSH)
                )
            )
        in_maps.append(m)
    return in_maps, has_bout


def run(inputs: dict, trace: bool = False):
    from concourse.bass_utils import run_bass_kernel_spmd

    in_maps, has_bout = _prep(**inputs)
    key = ("nc", has_bout)
    if key not in _cache:
        _cache[key] = build(has_bout)
    nc = _cache[key]
    res = run_bass_kernel_spmd(nc, in_maps, list(range(NCORES)), trace=trace)

    logits = np.concatenate([res.results[c]["logits_out"] for c in range(NCORES)], axis=1)
    h_sparse = np.concatenate(
        [res.results[c]["hsp_out"] for c in range(NCORES)], axis=0
    ).reshape(B, S, HID)
    w_new = np.concatenate([res.results[c]["wnew_out"] for c in range(NCORES)], axis=0)
    return (logits, h_sparse, w_new), res


def kernel(**inputs):
    outs, _ = run(inputs, trace=False)
    return outs
